# revision 6
# baseline (speedup 1.0000x reference)
"""Sharded cosine-similarity kNN (k=5) + weighted centroid on 8 TRN2 NeuronCores.

Strategy (standard sharded kNN):
  - Shard the 500000x768 f32 database row-wise across 8 cores (62500 rows each,
    padded to 62976 = 123 tiles x 512 rows with copies of -query, which have
    cosine similarity exactly -1 and can never enter the top-k).
  - Each core streams its 192 MB shard from HBM once (memory-bound roofline).
    Per [128, 3072] tile (4 db rows per partition, contiguous DMA):
      * DVE tensor_tensor_reduce: dot(row, q) fused multiply+reduce
      * ACT activation(Square, accum_out): ||row||^2 fused square+reduce
  - Epilogue per core: dn=sqrt(n2), clamp eps, inv=1/dn, sims=dots*inv, then
    vector.max + max_index -> per-partition top-8 candidates (values+indices).
  - Host: gather 8x128x8 candidates, divide by ||q|| (order-preserving), global
    top-5, inverse-square-distance weights, weighted centroid gather.
"""

import numpy as np

import concourse.bass as bass
import concourse.mybir as mybir
from concourse.tile import TileContext
from concourse.bass_utils import run_bass_kernel_spmd

from tile_patch import split_sync_waits

N_CORES = 8
D = 768
N_ROWS = 500000
SHARD = N_ROWS // N_CORES  # 62500
G = 4                      # db rows per partition per tile
P = 128
TILE_ROWS = P * G          # 512
NT_FULL = -(-SHARD // TILE_ROWS)  # 123
FREE = G * D               # 3072
K = 5
COS_EPS = 1e-8
W_EPS = 1e-6

_f32 = mybir.dt.float32
_u32 = mybir.dt.uint32


def build_nc(nt: int = NT_FULL, db_bufs: int = 6, repeat: int = 1):
    ncols = nt * G
    nc = bass.Bass()
    db = nc.dram_tensor("db", [nt, P, FREE], _f32, kind="ExternalInput")
    qrep = nc.dram_tensor("qrep", [P, FREE], _f32, kind="ExternalInput")
    outv = nc.dram_tensor("outv", [P, 8], _f32, kind="ExternalOutput")
    outi = nc.dram_tensor("outi", [P, 8], _u32, kind="ExternalOutput")

    with TileContext(nc) as tc:
        with (
            tc.tile_pool(name="persist", bufs=1) as persist,
            tc.tile_pool(name="dbp", bufs=db_bufs) as dbp,
            tc.tile_pool(name="dv", bufs=3) as dvp,
            tc.tile_pool(name="da", bufs=3) as dap,
        ):
            import contextlib
            loop_ctx = tc.For_i(0, repeat, 1) if repeat > 1 else contextlib.nullcontext()
            with loop_ctx:
                _body(nc, tc, persist, dbp, dvp, dap, db, qrep, outv, outi, nt, ncols)
    split_sync_waits(nc)
    return nc


def _body(nc, tc, persist, dbp, dvp, dap, db, qrep, outv, outi, nt, ncols):
    if True:
        if True:
            qt = persist.tile([P, FREE], _f32, tag="qt")
            nc.sync.dma_start(qt[:], qrep[:])

            dots = persist.tile([P, ncols], _f32, tag="dots")
            n2 = persist.tile([P, ncols], _f32, tag="n2")

            for t in range(nt):
                sb = dbp.tile([P, FREE], _f32, tag="sb")
                nc.sync.dma_start(sb[:], db[t])
                for j in range(G):
                    col = t * G + j
                    sl = sb[:, j * D : (j + 1) * D]
                    tout = dvp.tile([P, D], _f32, tag="tout")
                    nc.vector.scalar_tensor_tensor(
                        out=tout[:],
                        in0=sl,
                        scalar=0.0,
                        in1=qt[:, j * D : (j + 1) * D],
                        op0=mybir.AluOpType.bypass,
                        op1=mybir.AluOpType.mult,
                        accum_out=dots[:, col : col + 1],
                    )
                    aout = dap.tile([P, D], _f32, tag="aout")
                    nc.scalar.activation(
                        out=aout[:],
                        in_=sl,
                        func=mybir.ActivationFunctionType.Square,
                        accum_out=n2[:, col : col + 1],
                    )

            # sims = dots / max(sqrt(n2), eps)
            dn = persist.tile([P, ncols], _f32, tag="dn")
            nc.scalar.sqrt(dn[:], n2[:])
            nc.vector.tensor_scalar_max(dn[:], dn[:], COS_EPS)
            inv = persist.tile([P, ncols], _f32, tag="inv")
            nc.vector.reciprocal(inv[:], dn[:])
            sims = persist.tile([P, ncols], _f32, tag="sims")
            nc.vector.tensor_mul(sims[:], dots[:], inv[:])

            vals8 = persist.tile([P, 8], _f32, tag="vals8")
            idx8 = persist.tile([P, 8], _u32, tag="idx8")
            nc.vector.max(vals8[:], sims[:])
            nc.vector.max_index(idx8[:], vals8[:], sims[:])

            nc.sync.dma_start(outv[:], vals8[:])
            nc.sync.dma_start(outi[:], idx8[:])


def _prep_inputs(query: np.ndarray, database: np.ndarray, nt: int, n_cores: int,
                 shard: int):
    """Build per-core input maps. Pads each shard to nt*TILE_ROWS rows with
    copies of -query (cosine similarity -1: never selected)."""
    q = np.ascontiguousarray(np.asarray(query, dtype=np.float32)).reshape(1, D)
    db = np.asarray(database, dtype=np.float32)
    pad_rows = nt * TILE_ROWS
    qrep = np.ascontiguousarray(np.tile(q, (P, G)))  # [128, 3072]
    in_maps = []
    for c in range(n_cores):
        sh = np.empty((pad_rows, D), dtype=np.float32)
        sh[:shard] = db[c * shard : (c + 1) * shard]
        sh[shard:] = -q
        in_maps.append({"db": sh.reshape(nt, P, FREE), "qrep": qrep})
    return in_maps


def _host_reduce(results, query: np.ndarray, database: np.ndarray, nt: int,
                 n_cores: int, shard: int) -> np.ndarray:
    q = np.asarray(query, dtype=np.float32).reshape(1, D)
    db = np.asarray(database, dtype=np.float32)

    vals = np.stack([r["outv"] for r in results])          # [C,128,8] dot/||row||
    cols = np.stack([r["outi"] for r in results]).astype(np.int64)  # [C,128,8]

    c_idx = np.arange(n_cores, dtype=np.int64)[:, None, None]
    p_idx = np.arange(P, dtype=np.int64)[None, :, None]
    t = cols // G
    j = cols % G
    shard_row = t * TILE_ROWS + p_idx * G + j
    gidx = c_idx * shard + shard_row

    valid = (shard_row < shard).ravel()
    v = vals.ravel()[valid]
    g = gidx.ravel()[valid]

    qn = max(float(np.linalg.norm(q.astype(np.float64))), COS_EPS)
    sims = v / np.float32(qn)

    top = np.argsort(-sims, kind="stable")[:K]
    s = sims[top].astype(np.float64)
    idx = g[top]

    d = 1.0 - s
    w = 1.0 / (d + W_EPS) ** 2
    w = w / w.sum()
    centroid = (w[None, :] @ db[idx].astype(np.float64)).astype(np.float32)
    return centroid  # [1, D]


def _run(query: np.ndarray, database: np.ndarray, trace: bool = False):
    nc = build_nc()
    in_maps = _prep_inputs(query, database, NT_FULL, N_CORES, SHARD)
    res = run_bass_kernel_spmd(
        nc, in_maps, core_ids=list(range(N_CORES)), trace=trace,
    )
    out = _host_reduce(res.results, query, database, NT_FULL, N_CORES, SHARD)
    return out, res


def kernel(query: np.ndarray, database: np.ndarray) -> np.ndarray:
    out, _ = _run(query, database, trace=False)
    return out
